# revision 5
# baseline (speedup 1.0000x reference)
"""Trainium2 Bass kernel for the ragged Expand op — matmul-expand v3.

Semantics (matches the TF Expand layer / jax reference):
  x          [16, 4096, 256] f32
  dimensions [16, 4096, 1]   int32 repeat counts in [0, 8)
  out        [16, T, 256]    f32 where T = max_b sum_s d[b,s]
  out[b, t]  = x[b, idx[b,t]] for t < totals[b] else 0

Strategy: pure batch data-parallel over 8 NeuronCores (2 examples/core).
The expansion is computed as one-hot matmuls on the PE array:
  out_chunk[t, :] = P^T[:, t] . x_block     (P one-hot, built on host)

Masks are uploaded as fp8e4 (0/1 exact; PE supports fp8 lhsT with fp16
rhs, verified exact on HW), x as fp16; PSUM accumulates f32, so the
result is exactly fp16(x) — rel err ~2e-4 « 2e-2 tol. Each 128-row
source block u gets a per-slot output quota Q_u = max over cores of its
expanded length, rounded into NCHB_u uniform chunks of CR_u rows, so
the SPMD program is data-independent across cores; the host compacts
the padded output. Pipeline per chunk:
  PE matmul -> PSUM bank (rotating, 8)
  DVE (banks 0-3) / ACT (banks 4-7) copy -> SBUF stage
  one DMA write per BLOCK, alternating the two HWDGE rings (SP/ACT)

HBM traffic/core: ~8.5MB read + ~33MB write. No gpsimd, no library
load, no descriptor generation, ~75 output DMAs.
"""

import numpy as np

B, S, D = 16, 4096, 256
NCORES = 8
EX_PER_CORE = B // NCORES  # 2
BLK = 128
NBLK = S // BLK  # 32 blocks per example
NU = EX_PER_CORE * NBLK  # 64 block-columns per core
NST = 8  # stage ring depth (blocks): deep write backlog keeps all 16 SDMA engines fed
NBANK = 8  # PSUM banks
DVE_BANKS = 6  # banks 0..5 copied by vector engine, 6..7 by scalar
XSL = [0, 4, 12, 24, 40, 64]  # x upload slice boundaries (blocks)
NXD = len(XSL) - 1
NMD = 7  # mask upload slices (first ones small)



def _plan(dimensions):
    """Shared (cross-core) geometry. Returns dict of static plan data."""
    d = dimensions[:, :, 0].astype(np.int64)  # [B,S]
    totals = d.sum(1)
    T = int(totals.max())
    csum0 = np.concatenate(
        [np.zeros((B, 1), np.int64), d.cumsum(1)], axis=1
    )  # [B,S+1]
    m = d.reshape(B, NBLK, BLK).sum(2)  # [B, NBLK]
    # per block-slot quota: max over cores of that slot's expanded length
    mq = m.reshape(NCORES, EX_PER_CORE * NBLK).max(0)  # [NU]
    nchb = np.maximum(1, -(-mq // 128))  # chunks per block
    # rows per chunk, rounded up to a multiple of 16: DMA spray distributes
    # the outer (partition) dim over the 16 SDMA engines evenly only when it
    # divides by 16 — odd partition counts wrap the excess onto engines 0-1
    cr = np.minimum(128, 16 * (-(-(-(-mq // nchb)) // 16)))
    qp = nchb * cr  # padded rows per block
    base = np.concatenate([[0], np.cumsum(qp)])  # [NU+1] out row offsets
    # global chunk list: (u, j) in block-major order
    chunks = [(u, j) for u in range(NU) for j in range(int(nchb[u]))]
    # mask column offset per chunk
    mcol = np.concatenate([[0], np.cumsum([int(cr[u]) for u, _ in chunks])])
    return dict(
        d=d, totals=totals, T=T, csum0=csum0, m=m, nchb=nchb, cr=cr,
        qp=qp, base=base, chunks=chunks, mcol=mcol,
    )


def build_program(plan):
    import concourse.bass as bass  # noqa: F401
    import concourse.bacc as bacc
    import concourse.mybir as mybir
    from contextlib import ExitStack

    nchb, cr, base, chunks, mcol = (
        plan["nchb"], plan["cr"], plan["base"], plan["chunks"], plan["mcol"],
    )
    NCH = len(chunks)
    TOTROWS = int(base[-1])
    TOTMC = int(mcol[-1])
    fp16 = mybir.dt.float16
    fp8 = mybir.dt.float8e4
    f32 = mybir.dt.float32

    nc = bacc.Bacc("TRN2", num_devices=NCORES, name="expand_mm3")
    xbf_t = nc.dram_tensor("xbf", [128, NU * 256], fp16, kind="ExternalInput")
    msk_t = nc.dram_tensor("msk", [128, TOTMC], fp8, kind="ExternalInput")
    out_t = nc.dram_tensor("out", [TOTROWS, D], f32, kind="ExternalOutput")

    # copy-engine bookkeeping: chunk c -> bank c%8; banks 0..DVE_BANKS-1 on DVE
    def veng(c):
        return c % NBANK < DVE_BANKS

    vcnt = np.cumsum([1 if veng(c) else 0 for c in range(NCH)])
    scnt = np.cumsum([0 if veng(c) else 1 for c in range(NCH)])
    # last chunk index of each block
    lastc = {}
    for c, (u, j) in enumerate(chunks):
        lastc[u] = c
    # mask dma slices: small first slices so the PE can start early while
    # the bulk of the masks still streams in
    fr = [0.0, 0.05, 0.15, 0.3, 0.5, 0.7, 0.85, 1.0]
    msl_bounds = [int(round(f * NCH)) for f in fr]
    # which mask slice a chunk belongs to
    def mslice_of(c):
        for k in range(NMD):
            if c < msl_bounds[k + 1]:
                return k
        return NMD - 1

    MAXCHB = int(max(nchb))  # stage slot width (chunks per block)

    with (
        nc.Block() as block,
        nc.sbuf_tensor("x_sb", [128, NU, 256], fp16) as x_sb,
        nc.sbuf_tensor("msk_sb", [128, TOTMC], fp8) as msk_sb,
        nc.sbuf_tensor("stage", [128, NST, MAXCHB, 256], f32) as stage,
        nc.psum_tensor("acc", [128, NBANK, 512], f32) as acc,
        nc.semaphore("pe") as pe,
        nc.semaphore("cpv") as cpv,
        nc.semaphore("cps") as cps,
        ExitStack() as _stack,
    ):
        iox = [_stack.enter_context(nc.semaphore(f"iox{k}")) for k in range(NXD)]  # noqa: ANT232
        iom = [_stack.enter_context(nc.semaphore(f"iom{k}")) for k in range(NMD)]  # noqa: ANT232
        wsl = [_stack.enter_context(nc.semaphore(f"wsl{s}")) for s in range(NST)]  # noqa: ANT232
        def emit_write(eng, u):
            # wait all chunks of block u copied to stage
            lc = lastc[u]
            eng.wait_ge(cpv, int(vcnt[lc]))
            eng.wait_ge(cps, int(scnt[lc]))
            nch_u, cr_u = int(nchb[u]), int(cr[u])
            # chunk j holds block rows {j, j+NCHB, ...}: partition p of the
            # stage maps to NCHB consecutive DRAM rows. Split each block into
            # two DMAs (2-3KB descriptors) so each transfer has enough packets
            # to occupy all 16 SDMA engines.
            dst = out_t.ap()[int(base[u]) : int(base[u + 1]), :].rearrange(
                "(p c) e -> p c e", p=cr_u
            )
            eng.dma_start(dst, stage[:cr_u, u % NST, :nch_u, :]).then_inc(
                wsl[u % NST], 16
            )

        def stage_guard(eng, u):
            # stage slot u%NST reused from block u-NST; its write is ordered
            # transitively through the copy waits
            if u >= NST:
                eng.wait_ge(wsl[u % NST], 16 * (u // NST))

        @block.sync
        def _(sy: bass.BassEngine):
            for i in range(NXD):
                u0, u1 = XSL[i], XSL[i + 1]
                sy.dma_start(
                    x_sb[:, u0:u1, :],
                    xbf_t.ap()[:, u0 * 256 : u1 * 256],
                ).then_inc(iox[i], 16)
                if i == 0:
                    # let the tiny first slice land before the bulk loads
                    # contend for bandwidth: PE can then start ~2us earlier
                    sy.wait_ge(iox[0], 16)
            for u in range(0, NU, 2):
                emit_write(sy, u)

        @block.scalar
        def _(sc: bass.BassEngine):
            for k in range(NMD):
                c0, c1 = msl_bounds[k], msl_bounds[k + 1]
                sc.dma_start(
                    msk_sb[:, int(mcol[c0]) : int(mcol[c1])],
                    msk_t.ap()[:, int(mcol[c0]) : int(mcol[c1])],
                ).then_inc(iom[k], 16)
                if k == 0:
                    sc.wait_ge(iom[0], 16)
            for c, (u, j) in enumerate(chunks):
                if not veng(c):
                    bank = c % NBANK
                    cr_u = int(cr[u])
                    sc.wait_ge(pe, c + 1)
                    stage_guard(sc, u)
                    sc.copy(
                        stage[:cr_u, u % NST, j, :], acc[:cr_u, bank, :256]
                    ).then_inc(cps, 1)
                # interleave this block's write right after its last copy so
                # later stage-guard waits can't deadlock on a not-yet-issued
                # write further down this same instruction stream
                if c == lastc[u] and u % 2 == 1:
                    emit_write(sc, u)

        @block.tensor
        def _(te: bass.BassEngine):
            iox_seen = 0
            iom_seen = 0
            for c, (u, j) in enumerate(chunks):
                bank = c % NBANK
                cr_u = int(cr[u])
                nx = next(i for i in range(NXD) if u < XSL[i + 1]) + 1
                if nx > iox_seen:
                    te.wait_ge(iox[nx - 1], 16)
                    iox_seen = nx
                nm = mslice_of(c) + 1
                if nm > iom_seen:
                    te.wait_ge(iom[nm - 1], 16)
                    iom_seen = nm
                if c >= NBANK:
                    cp = c - NBANK
                    if veng(cp):
                        te.wait_ge(cpv, int(vcnt[cp]))
                    else:
                        te.wait_ge(cps, int(scnt[cp]))
                te.matmul(
                    acc[:cr_u, bank, :256],
                    msk_sb[:, int(mcol[c]) : int(mcol[c]) + cr_u],
                    x_sb[:, u, :],
                    start=True,
                    stop=True,
                ).then_inc(pe, 1)

        @block.vector
        def _(v: bass.BassEngine):
            for c, (u, j) in enumerate(chunks):
                if not veng(c):
                    continue
                bank = c % NBANK
                cr_u = int(cr[u])
                v.wait_ge(pe, c + 1)
                stage_guard(v, u)
                v.tensor_copy(
                    stage[:cr_u, u % NST, j, :], acc[:cr_u, bank, :256]
                ).then_inc(cpv, 1)

    nc.compile()
    return nc


def _install_ntff_hook():
    """Provide the antenv.axon_hooks module bass_utils expects for NTFF
    tracing under axon (the agent image ships without it)."""
    import sys
    import types

    if "antenv.axon_hooks" in sys.modules:
        return
    from trn_agent_boot.trn_boot import _ntff_profile_via_ctypes

    hook = _ntff_profile_via_ctypes("/opt/axon/libaxon_pjrt.so")
    mod = types.ModuleType("antenv.axon_hooks")
    state = {"hook": hook}
    mod.get_axon_ntff_profile_hook = lambda: state["hook"]
    mod.set_axon_ntff_profile_hook = lambda h: state.update(hook=h)
    sys.modules["antenv.axon_hooks"] = mod


def kernel(x, dimensions, _trace=False):
    import ml_dtypes

    x = np.ascontiguousarray(np.asarray(x), dtype=np.float32)
    dimensions = np.asarray(dimensions).astype(np.int32)

    plan = _plan(dimensions)
    d, csum0, m, nchb, cr, qp, base, chunks, mcol, T = (
        plan["d"], plan["csum0"], plan["m"], plan["nchb"], plan["cr"],
        plan["qp"], plan["base"], plan["chunks"], plan["mcol"], plan["T"],
    )
    TOTMC = int(mcol[-1])

    x16 = x.astype(np.float16)
    sidx = np.arange(BLK)[:, None]  # [128,1]

    in_maps = []
    for core in range(NCORES):
        xbf = np.empty((128, NU * 256), np.float16)
        msk = np.zeros((128, TOTMC), ml_dtypes.float8_e4m3)
        for e in range(EX_PER_CORE):
            b = EX_PER_CORE * core + e
            for k in range(NBLK):
                u = e * NBLK + k
                xbf[:, u * 256 : (u + 1) * 256] = x16[b, k * BLK : (k + 1) * BLK]
                mk = int(m[b, k])
                qp_u = int(qp[u])
                # source index per output row of this block (-1 = padding)
                srcof = np.full(qp_u, -1, np.int64)
                srcof[:mk] = np.repeat(
                    np.arange(BLK), d[b, k * BLK : (k + 1) * BLK]
                )
                onehot = (srcof[None, :] == sidx).astype(ml_dtypes.float8_e4m3)
                # [128, qp_u] -> chunk columns; chunk j computes the strided
                # rows {j, j+nch, j+2*nch, ...} so the staged block is
                # partition-major in DRAM row order (see emit_write)
                nch_u, cr_u = int(nchb[u]), int(cr[u])
                cu0 = int(np.cumsum(np.concatenate([[0], nchb]))[u])
                for j in range(nch_u):
                    c = cu0 + j
                    msk[:, int(mcol[c]) : int(mcol[c]) + cr_u] = onehot[
                        :, j::nch_u
                    ]
        in_maps.append({"xbf": xbf, "msk": msk})

    nc = build_program(plan)

    import concourse.bass_utils as bass_utils

    if _trace:
        _install_ntff_hook()
        bass_utils.upload_artifacts = lambda tmpdir: tmpdir

    res = bass_utils.run_bass_kernel_spmd(
        nc, in_maps, core_ids=list(range(NCORES)), trace=_trace
    )

    out = np.zeros((B, T, D), np.float32)
    for core in range(NCORES):
        st = res.results[core]["out"]  # [TOTROWS, 256]
        for e in range(EX_PER_CORE):
            b = EX_PER_CORE * core + e
            for k in range(NBLK):
                u = e * NBLK + k
                mk = int(m[b, k])
                if mk == 0:
                    continue
                ob = int(csum0[b, k * BLK])
                out[b, ob : ob + mk] = st[int(base[u]) : int(base[u]) + mk]
    if _trace:
        kernel.last_results = res
    return out


# revision 6
# speedup vs baseline: 1.1669x; 1.1669x over previous
"""Trainium2 Bass kernel for the ragged Expand op — matmul-expand v3.

Semantics (matches the TF Expand layer / jax reference):
  x          [16, 4096, 256] f32
  dimensions [16, 4096, 1]   int32 repeat counts in [0, 8)
  out        [16, T, 256]    f32 where T = max_b sum_s d[b,s]
  out[b, t]  = x[b, idx[b,t]] for t < totals[b] else 0

Strategy: pure batch data-parallel over 8 NeuronCores (2 examples/core).
The expansion is computed as one-hot matmuls on the PE array:
  out_chunk[t, :] = P^T[:, t] . x_block     (P one-hot, built on host)

Masks are uploaded as fp8e4 (0/1 exact; PE supports fp8 lhsT with fp16
rhs, verified exact on HW), x as fp16; PSUM accumulates f32, so the
result is exactly fp16(x) — rel err ~2e-4 « 2e-2 tol. Each 128-row
source block u gets a per-slot output quota Q_u = max over cores of its
expanded length, rounded into NCHB_u uniform chunks of CR_u rows, so
the SPMD program is data-independent across cores; the host compacts
the padded output. Pipeline per chunk:
  PE matmul -> PSUM bank (rotating, 8)
  DVE (banks 0-3) / ACT (banks 4-7) copy -> SBUF stage
  one DMA write per BLOCK, alternating the two HWDGE rings (SP/ACT)

HBM traffic/core: ~8.5MB read + ~33MB write. No gpsimd, no library
load, no descriptor generation, ~75 output DMAs.
"""

import numpy as np

B, S, D = 16, 4096, 256
NCORES = 8
EX_PER_CORE = B // NCORES  # 2
BLK = 128
NBLK = S // BLK  # 32 blocks per example
NU = EX_PER_CORE * NBLK  # 64 block-columns per core
NST = 8  # stage ring depth (blocks): deep write backlog keeps all 16 SDMA engines fed
NBANK = 8  # PSUM banks
DVE_BANKS = 6  # banks 0..5 copied by vector engine, 6..7 by scalar
XSL = [0, 4, 12, 24, 40, 64]  # x upload slice boundaries (blocks)
NXD = len(XSL) - 1
NMD = 7  # mask upload slices (first ones small)



def _plan(dimensions):
    """Shared (cross-core) geometry. Returns dict of static plan data."""
    d = dimensions[:, :, 0].astype(np.int64)  # [B,S]
    totals = d.sum(1)
    T = int(totals.max())
    csum0 = np.concatenate(
        [np.zeros((B, 1), np.int64), d.cumsum(1)], axis=1
    )  # [B,S+1]
    m = d.reshape(B, NBLK, BLK).sum(2)  # [B, NBLK]
    # per block-slot quota: max over cores of that slot's expanded length
    mq = m.reshape(NCORES, EX_PER_CORE * NBLK).max(0)  # [NU]
    nchb = np.maximum(1, -(-mq // 128))  # chunks per block
    # rows per chunk, rounded up to a multiple of 16: DMA spray distributes
    # the outer (partition) dim over the 16 SDMA engines evenly only when it
    # divides by 16 — odd partition counts wrap the excess onto engines 0-1
    cr = np.minimum(128, 16 * (-(-(-(-mq // nchb)) // 16)))
    qp = nchb * cr  # padded rows per block
    base = np.concatenate([[0], np.cumsum(qp)])  # [NU+1] out row offsets
    # global chunk list: (u, j) in block-major order
    chunks = [(u, j) for u in range(NU) for j in range(int(nchb[u]))]
    # mask column offset per chunk
    mcol = np.concatenate([[0], np.cumsum([int(cr[u]) for u, _ in chunks])])
    return dict(
        d=d, totals=totals, T=T, csum0=csum0, m=m, nchb=nchb, cr=cr,
        qp=qp, base=base, chunks=chunks, mcol=mcol,
    )


def build_program(plan):
    import concourse.bass as bass  # noqa: F401
    import concourse.bacc as bacc
    import concourse.mybir as mybir
    from contextlib import ExitStack

    nchb, cr, base, chunks, mcol = (
        plan["nchb"], plan["cr"], plan["base"], plan["chunks"], plan["mcol"],
    )
    NCH = len(chunks)
    TOTROWS = int(base[-1])
    TOTMC = int(mcol[-1])
    fp16 = mybir.dt.float16
    fp8 = mybir.dt.float8e4
    f32 = mybir.dt.float32

    nc = bacc.Bacc("TRN2", num_devices=NCORES, name="expand_mm3")
    xbf_t = nc.dram_tensor("xbf", [128, NU * 256], fp16, kind="ExternalInput")
    msk_t = nc.dram_tensor("msk", [128, TOTMC], fp8, kind="ExternalInput")
    out_t = nc.dram_tensor("out", [TOTROWS, D], f32, kind="ExternalOutput")

    # copy-engine bookkeeping: chunk c -> bank c%8; banks 0..DVE_BANKS-1 on DVE
    def veng(c):
        return c % NBANK < DVE_BANKS

    vcnt = np.cumsum([1 if veng(c) else 0 for c in range(NCH)])
    scnt = np.cumsum([0 if veng(c) else 1 for c in range(NCH)])
    # last chunk index of each block
    lastc = {}
    for c, (u, j) in enumerate(chunks):
        lastc[u] = c
    # mask dma slices: small first slices so the PE can start early while
    # the bulk of the masks still streams in
    fr = [0.0, 0.05, 0.15, 0.3, 0.5, 0.7, 0.85, 1.0]
    msl_bounds = [int(round(f * NCH)) for f in fr]
    # which mask slice a chunk belongs to
    def mslice_of(c):
        for k in range(NMD):
            if c < msl_bounds[k + 1]:
                return k
        return NMD - 1

    MAXCHB = int(max(nchb))  # stage slot width (chunks per block)

    with (
        nc.Block() as block,
        nc.sbuf_tensor("x_sb", [128, NU, 256], fp16) as x_sb,
        nc.sbuf_tensor("msk_sb", [128, TOTMC], fp8) as msk_sb,
        nc.sbuf_tensor("stage", [128, NST, MAXCHB, 256], f32) as stage,
        nc.psum_tensor("acc", [128, NBANK, 512], f32) as acc,
        nc.semaphore("pe") as pe,
        nc.semaphore("cpv") as cpv,
        nc.semaphore("cps") as cps,
        ExitStack() as _stack,
    ):
        iox = [_stack.enter_context(nc.semaphore(f"iox{k}")) for k in range(NXD)]  # noqa: ANT232
        iom = [_stack.enter_context(nc.semaphore(f"iom{k}")) for k in range(NMD)]  # noqa: ANT232
        wsl = [_stack.enter_context(nc.semaphore(f"wsl{s}")) for s in range(NST)]  # noqa: ANT232
        def emit_write(eng, u):
            # wait all chunks of block u copied to stage
            lc = lastc[u]
            eng.wait_ge(cpv, int(vcnt[lc]))
            eng.wait_ge(cps, int(scnt[lc]))
            nch_u, cr_u = int(nchb[u]), int(cr[u])
            # chunk j holds block rows {j, j+NCHB, ...}: partition p of the
            # stage maps to NCHB consecutive DRAM rows. Split each block into
            # two DMAs (2-3KB descriptors) so each transfer has enough packets
            # to occupy all 16 SDMA engines.
            dst = out_t.ap()[int(base[u]) : int(base[u + 1]), :].rearrange(
                "(p c) e -> p c e", p=cr_u
            )
            eng.dma_start(dst, stage[:cr_u, u % NST, :nch_u, :]).then_inc(
                wsl[u % NST], 16
            )

        def stage_guard(eng, u):
            # stage slot u%NST reused from block u-NST; its write is ordered
            # transitively through the copy waits
            if u >= NST:
                eng.wait_ge(wsl[u % NST], 16 * (u // NST))

        @block.sync
        def _(sy: bass.BassEngine):
            for i in range(NXD):
                u0, u1 = XSL[i], XSL[i + 1]
                sy.dma_start(
                    x_sb[:, u0:u1, :],
                    xbf_t.ap()[:, u0 * 256 : u1 * 256],
                ).then_inc(iox[i], 16)
            for u in range(0, NU, 2):
                emit_write(sy, u)

        @block.scalar
        def _(sc: bass.BassEngine):
            for k in range(NMD):
                c0, c1 = msl_bounds[k], msl_bounds[k + 1]
                sc.dma_start(
                    msk_sb[:, int(mcol[c0]) : int(mcol[c1])],
                    msk_t.ap()[:, int(mcol[c0]) : int(mcol[c1])],
                ).then_inc(iom[k], 16)
            for c, (u, j) in enumerate(chunks):
                if not veng(c):
                    bank = c % NBANK
                    cr_u = int(cr[u])
                    sc.wait_ge(pe, c + 1)
                    stage_guard(sc, u)
                    sc.copy(
                        stage[:cr_u, u % NST, j, :], acc[:cr_u, bank, :256]
                    ).then_inc(cps, 1)
                # interleave this block's write right after its last copy so
                # later stage-guard waits can't deadlock on a not-yet-issued
                # write further down this same instruction stream
                if c == lastc[u] and u % 2 == 1:
                    emit_write(sc, u)

        @block.tensor
        def _(te: bass.BassEngine):
            iox_seen = 0
            iom_seen = 0
            for c, (u, j) in enumerate(chunks):
                bank = c % NBANK
                cr_u = int(cr[u])
                nx = next(i for i in range(NXD) if u < XSL[i + 1]) + 1
                if nx > iox_seen:
                    te.wait_ge(iox[nx - 1], 16)
                    iox_seen = nx
                nm = mslice_of(c) + 1
                if nm > iom_seen:
                    te.wait_ge(iom[nm - 1], 16)
                    iom_seen = nm
                if c >= NBANK:
                    cp = c - NBANK
                    if veng(cp):
                        te.wait_ge(cpv, int(vcnt[cp]))
                    else:
                        te.wait_ge(cps, int(scnt[cp]))
                te.matmul(
                    acc[:cr_u, bank, :256],
                    msk_sb[:, int(mcol[c]) : int(mcol[c]) + cr_u],
                    x_sb[:, u, :],
                    start=True,
                    stop=True,
                ).then_inc(pe, 1)

        @block.vector
        def _(v: bass.BassEngine):
            for c, (u, j) in enumerate(chunks):
                if not veng(c):
                    continue
                bank = c % NBANK
                cr_u = int(cr[u])
                v.wait_ge(pe, c + 1)
                stage_guard(v, u)
                v.tensor_copy(
                    stage[:cr_u, u % NST, j, :], acc[:cr_u, bank, :256]
                ).then_inc(cpv, 1)

    nc.compile()
    return nc


def _install_ntff_hook():
    """Provide the antenv.axon_hooks module bass_utils expects for NTFF
    tracing under axon (the agent image ships without it)."""
    import sys
    import types

    if "antenv.axon_hooks" in sys.modules:
        return
    from trn_agent_boot.trn_boot import _ntff_profile_via_ctypes

    hook = _ntff_profile_via_ctypes("/opt/axon/libaxon_pjrt.so")
    mod = types.ModuleType("antenv.axon_hooks")
    state = {"hook": hook}
    mod.get_axon_ntff_profile_hook = lambda: state["hook"]
    mod.set_axon_ntff_profile_hook = lambda h: state.update(hook=h)
    sys.modules["antenv.axon_hooks"] = mod


def kernel(x, dimensions, _trace=False):
    import ml_dtypes

    x = np.ascontiguousarray(np.asarray(x), dtype=np.float32)
    dimensions = np.asarray(dimensions).astype(np.int32)

    plan = _plan(dimensions)
    d, csum0, m, nchb, cr, qp, base, chunks, mcol, T = (
        plan["d"], plan["csum0"], plan["m"], plan["nchb"], plan["cr"],
        plan["qp"], plan["base"], plan["chunks"], plan["mcol"], plan["T"],
    )
    TOTMC = int(mcol[-1])

    x16 = x.astype(np.float16)
    sidx = np.arange(BLK)[:, None]  # [128,1]

    in_maps = []
    for core in range(NCORES):
        xbf = np.empty((128, NU * 256), np.float16)
        msk = np.zeros((128, TOTMC), ml_dtypes.float8_e4m3)
        for e in range(EX_PER_CORE):
            b = EX_PER_CORE * core + e
            for k in range(NBLK):
                u = e * NBLK + k
                xbf[:, u * 256 : (u + 1) * 256] = x16[b, k * BLK : (k + 1) * BLK]
                mk = int(m[b, k])
                qp_u = int(qp[u])
                # source index per output row of this block (-1 = padding)
                srcof = np.full(qp_u, -1, np.int64)
                srcof[:mk] = np.repeat(
                    np.arange(BLK), d[b, k * BLK : (k + 1) * BLK]
                )
                onehot = (srcof[None, :] == sidx).astype(ml_dtypes.float8_e4m3)
                # [128, qp_u] -> chunk columns; chunk j computes the strided
                # rows {j, j+nch, j+2*nch, ...} so the staged block is
                # partition-major in DRAM row order (see emit_write)
                nch_u, cr_u = int(nchb[u]), int(cr[u])
                cu0 = int(np.cumsum(np.concatenate([[0], nchb]))[u])
                for j in range(nch_u):
                    c = cu0 + j
                    msk[:, int(mcol[c]) : int(mcol[c]) + cr_u] = onehot[
                        :, j::nch_u
                    ]
        in_maps.append({"xbf": xbf, "msk": msk})

    nc = build_program(plan)

    import concourse.bass_utils as bass_utils

    if _trace:
        _install_ntff_hook()
        bass_utils.upload_artifacts = lambda tmpdir: tmpdir

    res = bass_utils.run_bass_kernel_spmd(
        nc, in_maps, core_ids=list(range(NCORES)), trace=_trace
    )

    out = np.zeros((B, T, D), np.float32)
    for core in range(NCORES):
        st = res.results[core]["out"]  # [TOTROWS, 256]
        for e in range(EX_PER_CORE):
            b = EX_PER_CORE * core + e
            for k in range(NBLK):
                u = e * NBLK + k
                mk = int(m[b, k])
                if mk == 0:
                    continue
                ob = int(csum0[b, k * BLK])
                out[b, ob : ob + mk] = st[int(base[u]) : int(base[u]) + mk]
    if _trace:
        kernel.last_results = res
    return out
